# revision 11
# baseline (speedup 1.0000x reference)
"""LayerNorm-GRU (nn_Encoder_Base) Trainium2 Bass kernel, v3.

Contract: kernel(**inputs) takes FULL inputs (x [256,128,1024], W/U [1024,3072],
b [3072], gammas/betas [2,3072]) and returns the full output [256,128,1024].

Strategy: data-parallel over batch across 8 NeuronCores (32 rows/core, no
collectives).  All matmuls bf16 (1 PE cycle/row); all elementwise math f32
(DVE runs 1x regardless, so f32 costs nothing and keeps rel-err ~1.1e-2).
  Phase A: S1' = LN0(x@W+b) for all timesteps -> DRAM, gate pre-scale folded.
  Phase B: 128 sequential GRU steps; batch-stationary recurrent matmuls
  (lhsT = h^T chunks [128,32] bf16).  1/std comes from a DVE bit-trick rsqrt
  + 2 Newton steps, keeping the Act engine on a single function table
  (Square/Relu/Identity/Tanh) -- no ACT_TABLE_LOADs.  Gates use Relu(x) and
  Relu(1-x) with min(.,1) folded into the following multiply.  The r-path and
  the candidate tail are processed in 512-column halves so DVE/Act/PE
  pipeline; the f32 state add runs on GpSimd off the critical path.
"""

import numpy as np

_B, _T, _H = 256, 128, 1024
_ZR = 2 * _H          # 2048
_IN3 = 3 * _H         # 3072
_NCORES = 8
_BL = _B // _NCORES   # 32
_EPS = 1e-5
_TBLK = 4             # timesteps per phase-A tile (128 rows = 32 b * 4 t)
_MAGIC = 0x5F3759DF

_CACHE = {}


def _build(affine, has_bias, n_steps):
    from concourse import bacc
    import concourse.tile as tile
    import concourse.mybir as mybir
    from concourse.masks import make_identity

    from contextlib import ExitStack

    f32 = mybir.dt.float32
    bf16 = mybir.dt.bfloat16
    u32 = mybir.dt.uint32
    Alu = mybir.AluOpType
    Act = mybir.ActivationFunctionType
    Ax = mybir.AxisListType

    nc = bacc.Bacc("TRN2", target_bir_lowering=False, debug=False,
                   enable_asserts=False, num_devices=_NCORES)

    x_d = nc.dram_tensor("x", [_BL, n_steps, _H], bf16, kind="ExternalInput")
    w_d = nc.dram_tensor("w", [_H, _IN3 + 1], bf16, kind="ExternalInput")
    u_d = nc.dram_tensor("u", [_H, _IN3 + 2], bf16, kind="ExternalInput")
    o_d = nc.dram_tensor("o", [_BL, n_steps, _H], f32, kind="ExternalOutput")
    s1_d = nc.dram_tensor("s1", [_BL, n_steps, _IN3], f32)
    if has_bias:
        bias_d = nc.dram_tensor("bias", [_IN3], f32, kind="ExternalInput")
        bsum_d = nc.dram_tensor("bsum", [1], f32, kind="ExternalInput")
    if affine:
        pas_d = nc.dram_tensor("pas", [_IN3], f32, kind="ExternalInput")
        pab_d = nc.dram_tensor("pab", [_IN3], f32, kind="ExternalInput")
        g1_d = nc.dram_tensor("g1", [_IN3], f32, kind="ExternalInput")

    def bcast(vec_ap, p=128):
        import concourse.bass as bass
        return bass.AP(tensor=vec_ap.tensor, offset=vec_ap.offset,
                       ap=[[0, p]] + vec_ap.ap)

    with tile.TileContext(nc) as tc, ExitStack() as stack:
        persist = stack.enter_context(tc.tile_pool(name="persist", bufs=1))
        ident_b = persist.tile([128, 128], bf16, tag="ident_b")
        make_identity(nc, ident_b)
        magic = persist.tile([128, 1], u32, tag="magic")
        nc.vector.memset(magic, _MAGIC)

        u_sb = persist.tile([128, 8, _IN3 + 2], bf16, tag="u_sb")
        nc.sync.dma_start(out=u_sb,
                          in_=u_d.ap().rearrange("(k p) n -> p k n", p=128))

        h_sb = persist.tile([32, _H], f32, tag="h_sb")
        nc.vector.memzero(h_sb)
        h_bf = persist.tile([32, _H], bf16, tag="h_bf")
        nc.vector.memset(h_bf, 0.0)
        hT_sb = persist.tile([128, 8, 32], bf16, tag="hT_sb")
        nc.vector.memset(hT_sb, 0.0)

        def rsqrt_dve(sm, p, n, ssq, sums, inv_n, tag, extra_bias=None):
            """rinv = 1/sqrt(ssq/n - m^2 + eps), m = sums/n (+extra_bias).
            All on DVE: bit-trick seed + 2 Newton steps.  Returns (m, rinv)."""
            m = sm.tile([p, 1], f32, tag=f"{tag}_m")
            if extra_bias is not None:
                nc.vector.scalar_tensor_tensor(
                    out=m, in0=sums, scalar=inv_n, in1=extra_bias,
                    op0=Alu.mult, op1=Alu.add)
            else:
                nc.vector.tensor_scalar_mul(m, sums, inv_n)
            m2 = sm.tile([p, 1], f32, tag=f"{tag}_m2")
            nc.vector.tensor_mul(m2, m, m)
            nc.vector.tensor_scalar_add(m2, m2, -_EPS)
            var = sm.tile([p, 1], f32, tag=f"{tag}_var")
            nc.vector.scalar_tensor_tensor(
                out=var, in0=ssq, scalar=inv_n, in1=m2,
                op0=Alu.mult, op1=Alu.subtract)
            y = sm.tile([p, 1], f32, tag=f"{tag}_y")
            t = sm.tile([p, 1], f32, tag=f"{tag}_t")
            nc.vector.tensor_scalar(y.bitcast(u32), var.bitcast(u32),
                                    1, None, Alu.logical_shift_right)
            nc.vector.tensor_sub(y.bitcast(u32), magic[:p], y.bitcast(u32))
            for _ in range(2):
                nc.vector.tensor_mul(t, y, y)
                nc.vector.tensor_mul(t, t, var)
                nc.vector.tensor_scalar(t, t, -0.5, 1.5, Alu.mult, Alu.add)
                nc.vector.tensor_mul(y, y, t)
            return m, y

        if affine:
            consts = stack.enter_context(tc.tile_pool(name="consts", bufs=1))
            pas_bc = consts.tile([128, _IN3], f32)
            nc.sync.dma_start(out=pas_bc, in_=bcast(pas_d.ap()))
            pab_bc = consts.tile([128, _IN3], f32)
            nc.sync.dma_start(out=pab_bc, in_=bcast(pab_d.ap()))
            g1_bc = consts.tile([32, _IN3], f32)
            nc.sync.dma_start(out=g1_bc, in_=bcast(g1_d.ap(), p=32))

        # ---------------- Phase A ----------------
        assert n_steps % _TBLK == 0
        with tc.tile_pool(name="paw", bufs=1) as paw, \
             tc.tile_pool(name="pa", bufs=2) as pa, \
             tc.tile_pool(name="pa_sm", bufs=2) as sm, \
             tc.tile_pool(name="pa_ps", bufs=1, space="PSUM") as pps:
            w_sb = paw.tile([128, 8, _IN3 + 1], bf16, tag="w_sb")
            nc.sync.dma_start(out=w_sb,
                              in_=w_d.ap().rearrange("(k p) n -> p k n", p=128))
            if has_bias:
                b_bc = paw.tile([128, _IN3], f32, tag="b_bc")
                nc.sync.dma_start(out=b_bc, in_=bcast(bias_d.ap()))
                bsum_sb = paw.tile([128, 1], f32, tag="bsum_sb")
                nc.sync.dma_start(out=bsum_sb, in_=bcast(bsum_d.ap()))

            sq_scr = paw.tile([128, 512], f32, tag="sq_scr")

            for j in range(n_steps // _TBLK):
                t0 = j * _TBLK
                x_sb = pa.tile([128, _H], bf16, tag="x", bufs=3)
                nc.sync.dma_start(out=x_sb, in_=x_d.ap()[:, t0:t0 + _TBLK, :])
                psT = pps.tile([128, 4, 128], bf16, tag="psT")
                xT = pa.tile([128, 8, 128], bf16, tag="xT")
                for r in range(2):
                    for k in range(4):
                        kk = r * 4 + k
                        nc.tensor.transpose(
                            psT[:, k], x_sb[:, kk * 128:(kk + 1) * 128],
                            ident_b)
                    nc.vector.tensor_copy(out=xT[:, r * 4:(r + 1) * 4],
                                          in_=psT)

                ps_s = pps.tile([128, 1], f32, tag="ps_s")
                for k in range(8):
                    nc.tensor.matmul(ps_s, xT[:, k],
                                     w_sb[:, k, _IN3:_IN3 + 1],
                                     start=(k == 0), stop=(k == 7))
                ssq6 = sm.tile([128, 6], f32, tag="ssq6")
                chunks = []
                for n in range(6):
                    ps_n = pps.tile([128, 512], f32, tag=f"ch{n}")
                    chunks.append(ps_n)
                    for k in range(8):
                        nc.tensor.matmul(
                            ps_n, xT[:, k],
                            w_sb[:, k, n * 512:(n + 1) * 512],
                            start=(k == 0), stop=(k == 7))
                    if has_bias:
                        nc.vector.tensor_add(ps_n, ps_n,
                                             b_bc[:, n * 512:(n + 1) * 512])
                    nc.scalar.activation(out=sq_scr, in_=ps_n,
                                         func=Act.Square,
                                         accum_out=ssq6[:, n:n + 1])

                ssq = sm.tile([128, 1], f32, tag="ssq")
                nc.vector.tensor_reduce(out=ssq, in_=ssq6, axis=Ax.X,
                                        op=Alu.add)
                m, rinv = rsqrt_dve(
                    sm, 128, _IN3, ssq, ps_s, 1.0 / _IN3, "pa",
                    extra_bias=(bsum_sb if has_bias else None))

                s1o = pa.tile([128, _IN3], f32, tag="s1o")
                if affine:
                    nmr = sm.tile([128, 1], f32, tag="nmr")
                    nc.vector.scalar_tensor_tensor(
                        out=nmr, in0=m, scalar=-1.0, in1=rinv,
                        op0=Alu.mult, op1=Alu.mult)
                    for n in range(6):
                        sl = slice(n * 512, (n + 1) * 512)
                        nc.scalar.activation(out=s1o[:, sl], in_=chunks[n],
                                             func=Act.Identity, scale=rinv,
                                             bias=nmr)
                    nc.vector.tensor_mul(s1o, s1o, pas_bc)
                    nc.vector.tensor_add(s1o, s1o, pab_bc)
                else:
                    rinv02 = sm.tile([128, 1], f32, tag="rinv02")
                    nc.vector.tensor_scalar_mul(rinv02, rinv, 0.2)
                    bzr = sm.tile([128, 1], f32, tag="bzr")
                    nc.vector.scalar_tensor_tensor(
                        out=bzr, in0=m, scalar=-0.2, in1=rinv,
                        op0=Alu.mult, op1=Alu.mult)
                    nc.vector.tensor_scalar_add(bzr, bzr, 0.5)
                    bc = sm.tile([128, 1], f32, tag="bc")
                    nc.vector.scalar_tensor_tensor(
                        out=bc, in0=m, scalar=-1.0, in1=rinv,
                        op0=Alu.mult, op1=Alu.mult)
                    for n in range(6):
                        sl = slice(n * 512, (n + 1) * 512)
                        nc.scalar.activation(
                            out=s1o[:, sl], in_=chunks[n], func=Act.Identity,
                            scale=(rinv02 if n < 4 else rinv),
                            bias=(bzr if n < 4 else bc))
                nc.sync.dma_start(out=s1_d.ap()[:, t0:t0 + _TBLK, :], in_=s1o)

        # ---------------- Phase B ----------------
        id32_b = ident_b[:32, :32]
        with tc.tile_pool(name="pb", bufs=1) as pb, \
             tc.tile_pool(name="pb_sm", bufs=2) as sm, \
             tc.tile_pool(name="mm_ps", bufs=1, space="PSUM") as mm_ps, \
             tc.tile_pool(name="t_ps", bufs=1, space="PSUM") as t_ps:
            sq_scr = pb.tile([32, 512], f32, tag="sq_scr")
            HF = 512
            for t in range(n_steps):
                s1t = pb.tile([32, _IN3], f32, tag="s1t", bufs=3)
                nc.sync.dma_start(out=s1t, in_=s1_d.ap()[:, t, :])

                ps_z = mm_ps.tile([32, _H], f32, tag="pz")
                ps_r = mm_ps.tile([32, _H], f32, tag="pr")
                ps_sums = mm_ps.tile([32, 2], f32, tag="sums")
                for k in range(8):
                    nc.tensor.matmul(ps_sums[:, 0:1], hT_sb[:, k],
                                     u_sb[:, k, _IN3:_IN3 + 1],
                                     start=(k == 0), stop=(k == 7))
                ssq4 = sm.tile([32, 4], f32, tag="ssq4")
                for n in range(4):
                    dst = (ps_z if n < 2 else ps_r)[:, (n % 2) * 512:
                                                    (n % 2) * 512 + 512]
                    for k in range(8):
                        nc.tensor.matmul(
                            dst, hT_sb[:, k],
                            u_sb[:, k, n * 512:(n + 1) * 512],
                            start=(k == 0), stop=(k == 7))
                    nc.scalar.activation(out=sq_scr, in_=dst, func=Act.Square,
                                         accum_out=ssq4[:, n:n + 1])

                # LN stats for zr (N=2048), all-DVE tail
                ssq = sm.tile([32, 1], f32, tag="ssq")
                nc.vector.tensor_reduce(out=ssq, in_=ssq4, axis=Ax.X,
                                        op=Alu.add)
                m, rinv = rsqrt_dve(sm, 32, _ZR, ssq, ps_sums[:, 0:1],
                                    1.0 / _ZR, "zr")
                rinv02 = sm.tile([32, 1], f32, tag="rinv02")
                nc.vector.tensor_scalar_mul(rinv02, rinv, 0.2)
                nmr02 = sm.tile([32, 1], f32, tag="nmr02")
                nc.vector.scalar_tensor_tensor(
                    out=nmr02, in0=m, scalar=-0.2, in1=rinv,
                    op0=Alu.mult, op1=Alu.mult)
                bz1m = sm.tile([32, 1], f32, tag="bz1m")
                nc.vector.tensor_scalar(bz1m, nmr02, -1.0, 1.0,
                                        Alu.mult, Alu.add)

                # r path in 512-col halves: DVE t_r | Act Relu | DVE rh,
                # PE transposes and rhT copies pipelined per half
                t_r = pb.tile([32, _H], f32, tag="t_r")
                s_r = pb.tile([32, _H], f32, tag="s_r")
                rh = pb.tile([32, _H], bf16, tag="rh")
                psT = t_ps.tile([128, 8, 32], bf16, tag="T")
                rhT = pb.tile([128, 8, 32], bf16, tag="rhT")
                for hf in range(2):
                    sl = slice(hf * HF, (hf + 1) * HF)
                    if affine:
                        nc.vector.tensor_scalar(t_r[:, sl], ps_r[:, sl],
                                                rinv02, nmr02,
                                                Alu.mult, Alu.add)
                        nc.vector.tensor_mul(t_r[:, sl], t_r[:, sl],
                                             g1_bc[:, _H + hf * HF:
                                                   _H + (hf + 1) * HF])
                        nc.vector.tensor_add(t_r[:, sl], t_r[:, sl],
                                             s1t[:, _H + hf * HF:
                                                 _H + (hf + 1) * HF])
                    else:
                        nc.vector.scalar_tensor_tensor(
                            out=t_r[:, sl], in0=ps_r[:, sl], scalar=rinv02,
                            in1=s1t[:, _H + hf * HF:_H + (hf + 1) * HF],
                            op0=Alu.mult, op1=Alu.add)
                    nc.scalar.activation(out=s_r[:, sl], in_=t_r[:, sl],
                                         func=Act.Relu,
                                         bias=(0.0 if affine else nmr02))
                for hf in range(2):
                    sl = slice(hf * HF, (hf + 1) * HF)
                    nc.vector.scalar_tensor_tensor(
                        out=rh[:, sl], in0=s_r[:, sl], scalar=1.0,
                        in1=h_bf[:, sl], op0=Alu.min, op1=Alu.mult)
                    for k in range(4):
                        kk = hf * 4 + k
                        nc.tensor.transpose(
                            psT[:, kk], rh[:, kk * 128:(kk + 1) * 128],
                            id32_b)
                    nc.vector.tensor_copy(out=rhT[:, hf * 4:(hf + 1) * 4],
                                          in_=psT[:, hf * 4:(hf + 1) * 4])

                # z path (overlaps the candidate matmul); omz before s_z so
                # the rhT copy wins the DVE slot over t1
                t_z = pb.tile([32, _H], f32, tag="t_z")
                if affine:
                    nc.vector.tensor_scalar(t_z, ps_z, rinv02, nmr02,
                                            Alu.mult, Alu.add)
                    nc.vector.tensor_mul(t_z, t_z, g1_bc[:, :_H])
                    nc.vector.tensor_add(t_z, t_z, s1t[:, :_H])
                else:
                    nc.vector.scalar_tensor_tensor(
                        out=t_z, in0=ps_z, scalar=rinv02, in1=s1t[:, :_H],
                        op0=Alu.mult, op1=Alu.add)
                omz = pb.tile([32, _H], f32, tag="omz")
                nc.scalar.activation(out=omz, in_=t_z, func=Act.Relu,
                                     scale=-1.0,
                                     bias=(1.0 if affine else bz1m))
                s_z = pb.tile([32, _H], f32, tag="s_z")
                nc.scalar.activation(out=s_z, in_=t_z, func=Act.Relu,
                                     bias=(0.0 if affine else nmr02))
                t1 = pb.tile([32, _H], f32, tag="t1")
                nc.vector.scalar_tensor_tensor(
                    out=t1, in0=s_z, scalar=1.0, in1=h_sb,
                    op0=Alu.min, op1=Alu.mult)

                # candidate matmuls
                ps_c = mm_ps.tile([32, _H], f32, tag="pc")
                for k in range(8):
                    nc.tensor.matmul(ps_sums[:, 1:2], rhT[:, k],
                                     u_sb[:, k, _IN3 + 1:_IN3 + 2],
                                     start=(k == 0), stop=(k == 7))
                ssq2 = sm.tile([32, 2], f32, tag="ssq2")
                for n in range(2):
                    dst = ps_c[:, n * 512:(n + 1) * 512]
                    for k in range(8):
                        nc.tensor.matmul(
                            dst, rhT[:, k],
                            u_sb[:, k, _ZR + n * 512:_ZR + (n + 1) * 512],
                            start=(k == 0), stop=(k == 7))
                    nc.scalar.activation(out=sq_scr, in_=dst, func=Act.Square,
                                         accum_out=ssq2[:, n:n + 1])

                # LN stats for c (N=1024)
                ssq_c = sm.tile([32, 1], f32, tag="ssqc")
                nc.vector.tensor_reduce(out=ssq_c, in_=ssq2, axis=Ax.X,
                                        op=Alu.add)
                m_c, rinv_c = rsqrt_dve(sm, 32, _H, ssq_c, ps_sums[:, 1:2],
                                        1.0 / _H, "c")
                nmr_c = sm.tile([32, 1], f32, tag="nmrc")
                nc.vector.scalar_tensor_tensor(
                    out=nmr_c, in0=m_c, scalar=-1.0, in1=rinv_c,
                    op0=Alu.mult, op1=Alu.mult)

                # candidate tail + h update in halves:
                # tanh_a overlaps t_c_b; transposes/copies pipelined
                t_c = pb.tile([32, _H], f32, tag="t_c")
                cand = pb.tile([32, _H], f32, tag="cand")
                t2 = pb.tile([32, _H], f32, tag="t2")
                for hf in range(2):
                    sl = slice(hf * HF, (hf + 1) * HF)
                    if affine:
                        nc.vector.tensor_scalar(t_c[:, sl], ps_c[:, sl],
                                                rinv_c, nmr_c,
                                                Alu.mult, Alu.add)
                        nc.vector.tensor_mul(t_c[:, sl], t_c[:, sl],
                                             g1_bc[:, _ZR + hf * HF:
                                                   _ZR + (hf + 1) * HF])
                        nc.vector.tensor_add(t_c[:, sl], t_c[:, sl],
                                             s1t[:, _ZR + hf * HF:
                                                 _ZR + (hf + 1) * HF])
                    else:
                        nc.vector.scalar_tensor_tensor(
                            out=t_c[:, sl], in0=ps_c[:, sl], scalar=rinv_c,
                            in1=s1t[:, _ZR + hf * HF:_ZR + (hf + 1) * HF],
                            op0=Alu.mult, op1=Alu.add)
                    nc.scalar.activation(out=cand[:, sl], in_=t_c[:, sl],
                                         func=Act.Tanh,
                                         bias=(0.0 if affine else nmr_c))
                for hf in range(2):
                    sl = slice(hf * HF, (hf + 1) * HF)
                    nc.vector.scalar_tensor_tensor(
                        out=t2[:, sl], in0=omz[:, sl], scalar=1.0,
                        in1=cand[:, sl], op0=Alu.min, op1=Alu.mult)
                    nc.vector.tensor_add(h_bf[:, sl], t1[:, sl], t2[:, sl])
                    for k in range(4):
                        kk = hf * 4 + k
                        nc.tensor.transpose(
                            psT[:, kk], h_bf[:, kk * 128:(kk + 1) * 128],
                            id32_b)
                    nc.vector.tensor_copy(out=hT_sb[:, hf * 4:(hf + 1) * 4],
                                          in_=psT[:, hf * 4:(hf + 1) * 4])
                # f32 state update off the critical path, on GpSimd
                nc.gpsimd.tensor_add(h_sb, t1, t2)
                nc.sync.dma_start(out=o_d.ap()[:, t, :], in_=h_sb)

    nc.compile()
    return nc


def _get_nc(affine, has_bias, n_steps):
    key = (affine, has_bias, n_steps)
    if key not in _CACHE:
        _CACHE[key] = _build(affine, has_bias, n_steps)
    return _CACHE[key]


LAST_RESULTS = None


def kernel(x, W, U, b, gammas, betas, n_steps=_T, trace=False):
    global LAST_RESULTS
    import ml_dtypes
    from concourse.bass_utils import run_bass_kernel_spmd

    bf = ml_dtypes.bfloat16
    x = np.ascontiguousarray(np.asarray(x, dtype=np.float32))[:, :n_steps]
    W = np.asarray(W, dtype=np.float32)
    U = np.asarray(U, dtype=np.float32)
    b = np.asarray(b, dtype=np.float32)
    gammas = np.asarray(gammas, dtype=np.float32)
    betas = np.asarray(betas, dtype=np.float32)

    has_bias = bool(np.any(b != 0.0))
    affine = bool(np.any(gammas != 1.0) or np.any(betas != 0.0))

    w_ext = np.concatenate(
        [W, W.sum(1, keepdims=True, dtype=np.float64).astype(np.float32)],
        axis=1).astype(bf)
    u_ext = np.concatenate(
        [U,
         U[:, :_ZR].sum(1, keepdims=True, dtype=np.float64).astype(np.float32),
         U[:, _ZR:].sum(1, keepdims=True, dtype=np.float64).astype(np.float32)],
        axis=1).astype(bf)
    x_bf = x.astype(bf)

    nc = _get_nc(affine, has_bias, n_steps)

    in_maps = []
    for c in range(_NCORES):
        m = {"x": np.ascontiguousarray(x_bf[c * _BL:(c + 1) * _BL]),
             "w": w_ext, "u": u_ext}
        if has_bias:
            m["bias"] = b
            m["bsum"] = np.array([b.sum(dtype=np.float64) / _IN3],
                                 dtype=np.float32)
        if affine:
            g0, b0 = gammas[0], betas[0]
            g1, b1 = gammas[1], betas[1]
            pas = np.concatenate([0.2 * g0[:_ZR], g0[_ZR:]])
            pab = np.concatenate(
                [0.2 * b0[:_ZR] + 0.5 + 0.2 * b1[:_ZR], b0[_ZR:] + b1[_ZR:]])
            m["pas"] = pas.astype(np.float32)
            m["pab"] = pab.astype(np.float32)
            m["g1"] = g1.astype(np.float32)
        in_maps.append(m)

    res = run_bass_kernel_spmd(nc, in_maps, list(range(_NCORES)), trace=trace)
    LAST_RESULTS = res
    out = np.concatenate([res.results[c]["o"] for c in range(_NCORES)], axis=0)
    return out


# revision 19
# speedup vs baseline: 1.2488x; 1.2488x over previous
"""LayerNorm-GRU (nn_Encoder_Base) Trainium2 Bass kernel, v4.

Contract: kernel(**inputs) takes FULL inputs (x [256,128,1024], W/U [1024,3072],
b [3072], gammas/betas [2,3072]) and returns the full output [256,128,1024].

Strategy: data-parallel over batch across 8 NeuronCores (32 rows/core, no
collectives).  All matmuls bf16; elementwise math f32 (rel-err ~1.1e-2).

Phase B runs 128 sequential GRU steps with batch-stationary matmuls
(lhsT = h^T chunks [128,32] bf16).  Per 512-col chunk PSUM tiles avoid
false tile-granular WAR serialization; 1/std is a DVE bit-trick rsqrt with
Newton steps, mostly precomputed on 3/4 of the columns while the last matmul
chunk runs; the Act engine stays on one function table (Square/Relu/Tanh/
Identity).  Gates are Relu(x)/Relu(1-x) with min(.,1) folded into the next
multiply; the r-path and candidate tail are processed in 512-col halves.

Phase A (S1' = LN0(x@W+b), gate pre-scale folded) is interleaved into
phase B: its matmul chunks run in the PE gap after the candidate matmuls,
time-sharing the four zr PSUM banks ([128,512] tiles whose [:32] views are
the phase-B chunk outputs); x tiles are transposed by the DMA xbar; chunk
PSUM->SBUF staging + row-sums and Squares run on Act at the gap end; LN
applies are throttled to 2 per step.
"""

import numpy as np

_B, _T, _H = 256, 128, 1024
_ZR = 2 * _H          # 2048
_IN3 = 3 * _H         # 3072
_NCORES = 8
_BL = _B // _NCORES   # 32
_EPS = 1e-5
_TBLK = 4             # timesteps per phase-A tile (128 rows = 32 b * 4 t)
_MAGIC = 0x5F3759DF
_NBOOT = 2            # phase-A tiles emitted before step 0
_AMM_PER_STEP = 2     # phase-A matmul chunks per phase-B step
_APPLY_PER_STEP = 2   # phase-A LN applies per phase-B step

_CACHE = {}


def _build(affine, has_bias, n_steps):
    from concourse import bacc
    import concourse.tile as tile
    import concourse.mybir as mybir
    from concourse.masks import make_identity

    from contextlib import ExitStack

    f32 = mybir.dt.float32
    bf16 = mybir.dt.bfloat16
    u32 = mybir.dt.uint32
    Alu = mybir.AluOpType
    Act = mybir.ActivationFunctionType
    Ax = mybir.AxisListType

    nc = bacc.Bacc("TRN2", target_bir_lowering=False, debug=False,
                   enable_asserts=False, num_devices=_NCORES)

    x_d = nc.dram_tensor("x", [_BL, n_steps, _H], bf16, kind="ExternalInput")
    w_d = nc.dram_tensor("w", [_H, _IN3], bf16, kind="ExternalInput")
    u_d = nc.dram_tensor("u", [_H, _IN3 + 2], bf16, kind="ExternalInput")
    o_d = nc.dram_tensor("o", [_BL, n_steps, _H], f32, kind="ExternalOutput")
    s1_d = nc.dram_tensor("s1", [_BL, n_steps, _IN3], f32)
    if has_bias:
        bias_d = nc.dram_tensor("bias", [_IN3], f32, kind="ExternalInput")
        bsum_d = nc.dram_tensor("bsum", [1], f32, kind="ExternalInput")
    if affine:
        pas_d = nc.dram_tensor("pas", [_IN3], f32, kind="ExternalInput")
        pab_d = nc.dram_tensor("pab", [_IN3], f32, kind="ExternalInput")
        g1_d = nc.dram_tensor("g1", [_IN3], f32, kind="ExternalInput")

    def bcast(vec_ap, p=128):
        import concourse.bass as bass
        return bass.AP(tensor=vec_ap.tensor, offset=vec_ap.offset,
                       ap=[[0, p]] + vec_ap.ap)

    assert n_steps % _TBLK == 0
    n_tiles = n_steps // _TBLK

    with tile.TileContext(nc) as tc, ExitStack() as stack:
        persist = stack.enter_context(tc.tile_pool(name="persist", bufs=1))
        ident_b = persist.tile([32, 32], bf16, tag="ident_b")
        make_identity(nc, ident_b)
        magic = persist.tile([128, 1], u32, tag="magic")
        nc.vector.memset(magic, _MAGIC)

        u_sb = persist.tile([128, 8, _IN3 + 2], bf16, tag="u_sb")
        nc.sync.dma_start(out=u_sb,
                          in_=u_d.ap().rearrange("(k p) n -> p k n", p=128))
        w_sb = persist.tile([128, 8, _IN3], bf16, tag="w_sb")
        nc.sync.dma_start(out=w_sb,
                          in_=w_d.ap().rearrange("(k p) n -> p k n", p=128))

        h_sb = persist.tile([32, _H], f32, tag="h_sb")
        nc.vector.memzero(h_sb)
        h_bf = persist.tile([32, _H], bf16, tag="h_bf")
        nc.vector.memset(h_bf, 0.0)
        hT_a = persist.tile([128, 4, 32], bf16, tag="hT_a")
        nc.vector.memset(hT_a, 0.0)
        hT_b = persist.tile([128, 4, 32], bf16, tag="hT_b")
        nc.vector.memset(hT_b, 0.0)

        if has_bias:
            b_bc = persist.tile([128, _IN3], f32, tag="b_bc")
            nc.sync.dma_start(out=b_bc, in_=bcast(bias_d.ap()))
            bsum_sb = persist.tile([128, 1], f32, tag="bsum_sb")
            nc.sync.dma_start(out=bsum_sb, in_=bcast(bsum_d.ap()))
        if affine:
            pas_bc = persist.tile([128, _IN3], f32, tag="pas_bc")
            nc.sync.dma_start(out=pas_bc, in_=bcast(pas_d.ap()))
            pab_bc = persist.tile([128, _IN3], f32, tag="pab_bc")
            nc.sync.dma_start(out=pab_bc, in_=bcast(pab_d.ap()))
            g1_bc = persist.tile([32, _IN3], f32, tag="g1_bc")
            nc.sync.dma_start(out=g1_bc, in_=bcast(g1_d.ap(), p=32))

        apool = stack.enter_context(tc.tile_pool(name="apool", bufs=1))
        bpool = stack.enter_context(tc.tile_pool(name="bpool", bufs=1))
        smpool = stack.enter_context(tc.tile_pool(name="smpool", bufs=2))
        mm_ps = stack.enter_context(
            tc.tile_pool(name="mm_ps", bufs=1, space="PSUM"))
        t_ps = stack.enter_context(
            tc.tile_pool(name="t_ps", bufs=1, space="PSUM"))

        # ---------- small helpers ----------
        def newton_seed(sm, p, var_est, tag):
            """bit-trick seed + 2 NR iterations on an ESTIMATED variance."""
            y = sm.tile([p, 1], f32, tag=f"{tag}_y")
            t = sm.tile([p, 1], f32, tag=f"{tag}_t")
            nc.vector.tensor_scalar(y.bitcast(u32), var_est.bitcast(u32),
                                    1, None, Alu.logical_shift_right)
            nc.vector.tensor_sub(y.bitcast(u32), magic[:p], y.bitcast(u32))
            for _ in range(2):
                nc.vector.tensor_mul(t, y, y)
                nc.vector.tensor_mul(t, t, var_est)
                nc.vector.tensor_scalar(t, t, -0.5, 1.5, Alu.mult, Alu.add)
                nc.vector.tensor_mul(y, y, t)
            return y, t

        def newton_refine(y, t, var):
            """one NR step of y towards rsqrt(var) (t is scratch)."""
            nc.vector.tensor_mul(t, y, y)
            nc.vector.tensor_mul(t, t, var)
            nc.vector.tensor_scalar(t, t, -0.5, 1.5, Alu.mult, Alu.add)
            nc.vector.tensor_mul(y, y, t)

        # ---------- phase-A machinery (interleaved) ----------
        BANK_TAGS = ["bk0", "bk1", "bk2", "bk3"]
        a_state = {
            "pending": [(j, n) for j in range(n_tiles) for n in range(6)],
            "head": 0,
            "gidx": 0,              # global chunk counter for bank rotation
            "tiles": {},            # j -> dict of tiles
            "apply_queue": [],      # (j, n) applies not yet emitted
            "copied": {},           # j -> count of staged chunks
        }

        def a_tile_start(j):
            t0 = j * _TBLK
            x_sb = apool.tile([128, _H], bf16, tag="x_sb")
            nc.sync.dma_start(out=x_sb, in_=x_d.ap()[:, t0:t0 + _TBLK, :])
            xT = apool.tile([128, 8, 128], bf16, tag="xT", bufs=2)
            for k in range(8):
                nc.sync.dma_start(out=xT[:, k],
                                  in_=x_sb[:, k * 128:(k + 1) * 128],
                                  transpose=True)
            st = {
                "xT": xT,
                "pch": [None] * 6,
                "ssq6": smpool.tile([128, 6], f32, tag="a_ssq6", name="a_ssq6"),
                "sums6": smpool.tile([128, 6], f32, tag="a_sums6", name="a_sums6"),
                "s1o": apool.tile([128, _IN3], f32, tag="s1o", name="s1o"),
            }
            a_state["tiles"][j] = st
            a_state["copied"][j] = 0
            return st

        def a_emit_mm_pe(j, n):
            """PE: 8 matmuls of chunk n into a rotated zr bank."""
            st = a_state["tiles"].get(j) or a_tile_start(j)
            bank = mm_ps.tile([128, 512], f32,
                              tag=BANK_TAGS[a_state["gidx"] % 4])
            a_state["gidx"] += 1
            for k in range(8):
                nc.tensor.matmul(bank, st["xT"][:, k],
                                 w_sb[:, k, n * 512:(n + 1) * 512],
                                 start=(k == 0), stop=(k == 7))
            return bank

        def a_emit_mm_act(j, n, bank):
            """Act: stage PSUM->SBUF (with rowsum) and Square the copy."""
            st = a_state["tiles"][j]
            pch = apool.tile([128, 512], f32, tag=f"pch{n}", bufs=2)
            st["pch"][n] = pch
            nc.scalar.activation(out=pch, in_=bank, func=Act.Identity,
                                 accum_out=st["sums6"][:, n:n + 1])
            if has_bias:
                nc.vector.tensor_add(pch, pch,
                                     b_bc[:, n * 512:(n + 1) * 512])
            sl = slice(n * 512, (n + 1) * 512)
            nc.scalar.activation(out=st["s1o"][:, sl], in_=pch,
                                 func=Act.Square,
                                 accum_out=st["ssq6"][:, n:n + 1])
            a_state["copied"][j] += 1
            if a_state["copied"][j] == 6:
                a_emit_stats(j)

        def a_emit_stats(j):
            st = a_state["tiles"][j]
            sm = smpool
            ssq = sm.tile([128, 1], f32, tag="a_ssq")
            nc.vector.tensor_reduce(out=ssq, in_=st["ssq6"], axis=Ax.X,
                                    op=Alu.add)
            sums = sm.tile([128, 1], f32, tag="a_sums")
            nc.vector.tensor_reduce(out=sums, in_=st["sums6"], axis=Ax.X,
                                    op=Alu.add)
            m = sm.tile([128, 1], f32, tag="a_m")
            if has_bias:
                nc.vector.scalar_tensor_tensor(
                    out=m, in0=sums, scalar=1.0 / _IN3, in1=bsum_sb,
                    op0=Alu.mult, op1=Alu.add)
            else:
                nc.vector.tensor_scalar_mul(m, sums, 1.0 / _IN3)
            m2 = sm.tile([128, 1], f32, tag="a_m2")
            nc.vector.tensor_mul(m2, m, m)
            nc.vector.tensor_scalar_add(m2, m2, -_EPS)
            var = sm.tile([128, 1], f32, tag="a_var")
            nc.vector.scalar_tensor_tensor(
                out=var, in0=ssq, scalar=1.0 / _IN3, in1=m2,
                op0=Alu.mult, op1=Alu.subtract)
            y, t = newton_seed(sm, 128, var, "a")
            newton_refine(y, t, var)
            st["rinv"] = y
            if affine:
                nmr = sm.tile([128, 1], f32, tag="a_nmr")
                nc.vector.scalar_tensor_tensor(
                    out=nmr, in0=m, scalar=-1.0, in1=y,
                    op0=Alu.mult, op1=Alu.mult)
                st["sc"] = [y] * 6
                st["bi"] = [nmr] * 6
            else:
                rinv02 = sm.tile([128, 1], f32, tag="a_rinv02")
                nc.vector.tensor_scalar_mul(rinv02, y, 0.2)
                bzr = sm.tile([128, 1], f32, tag="a_bzr")
                nc.vector.scalar_tensor_tensor(
                    out=bzr, in0=m, scalar=-0.2, in1=y,
                    op0=Alu.mult, op1=Alu.mult)
                nc.vector.tensor_scalar_add(bzr, bzr, 0.5)
                bc_ = sm.tile([128, 1], f32, tag="a_bc")
                nc.vector.scalar_tensor_tensor(
                    out=bc_, in0=m, scalar=-1.0, in1=y,
                    op0=Alu.mult, op1=Alu.mult)
                st["sc"] = [rinv02] * 4 + [y] * 2
                st["bi"] = [bzr] * 4 + [bc_] * 2
            a_state["apply_queue"].extend((j, n) for n in range(6))

        def a_emit_apply(j, n):
            st = a_state["tiles"][j]
            sl = slice(n * 512, (n + 1) * 512)
            nc.scalar.activation(out=st["s1o"][:, sl], in_=st["pch"][n],
                                 func=Act.Identity, scale=st["sc"][n],
                                 bias=st["bi"][n])
            if affine:
                nc.vector.tensor_mul(st["s1o"][:, sl], st["s1o"][:, sl],
                                     pas_bc[:, sl])
                nc.vector.tensor_add(st["s1o"][:, sl], st["s1o"][:, sl],
                                     pab_bc[:, sl])
            if n == 5:
                t0 = j * _TBLK
                nc.sync.dma_start(out=s1_d.ap()[:, t0:t0 + _TBLK, :],
                                  in_=st["s1o"])
                st["pch"] = [None] * 6  # allow slot reuse

        def a_emit_chunks_pe(budget):
            done = []
            while budget > 0 and a_state["head"] < len(a_state["pending"]):
                j, n = a_state["pending"][a_state["head"]]
                a_state["head"] += 1
                done.append((j, n, a_emit_mm_pe(j, n)))
                budget -= 1
            return done

        def a_emit_chunks_act(done):
            for j, n, bank in done:
                a_emit_mm_act(j, n, bank)

        def a_emit_applies(budget):
            q = a_state["apply_queue"]
            while budget > 0 and q:
                j, n = q.pop(0)
                a_emit_apply(j, n)
                budget -= 1

        # ---------- bootstrap: first tiles fully, before step 0 ----------
        for j in range(_NBOOT):
            a_emit_chunks_act(a_emit_chunks_pe(6))
            a_emit_applies(6)

        # ---------- phase B ----------
        sq_scr = bpool.tile([32, 512], f32, tag="sq_scr")
        for t in range(n_steps):
            sm = smpool
            s1t = bpool.tile([32, _IN3], f32, tag="s1t", bufs=2)
            nc.sync.dma_start(out=s1t, in_=s1_d.ap()[:, t, :])

            zb = [mm_ps.tile([128, 512], f32, tag=BANK_TAGS[i],
                             name=BANK_TAGS[i])[:32]
                  for i in range(4)]
            ps_c0 = mm_ps.tile([32, 512], f32, tag="pc0")
            ps_c1 = mm_ps.tile([32, 512], f32, tag="pc1")
            ps_sums = mm_ps.tile([32, 2], f32, tag="sums")

            ssq4 = sm.tile([32, 4], f32, tag="ssq4")
            # z0 first (absorbs PE ramp), then the cheap row-sum matmuls so
            # the mean is ready for the partial-stats Newton.
            for n in range(4):
                dst = zb[n]
                for k in range(8):
                    hT = hT_a if k < 4 else hT_b
                    nc.tensor.matmul(
                        dst, hT[:, k % 4],
                        u_sb[:, k, n * 512:(n + 1) * 512],
                        start=(k == 0), stop=(k == 7))
                if n == 0:
                    for k in range(8):
                        hT = hT_a if k < 4 else hT_b
                        nc.tensor.matmul(ps_sums[:, 0:1], hT[:, k % 4],
                                         u_sb[:, k, _IN3:_IN3 + 1],
                                         start=(k == 0), stop=(k == 7))
                nc.scalar.activation(out=sq_scr, in_=dst, func=Act.Square,
                                     accum_out=ssq4[:, n:n + 1])
                if n == 2:
                    # partial stats on 3/4 of the columns, hidden under the
                    # r1 matmuls: mean, m^2-eps, est-variance, Newton seed
                    red3 = sm.tile([32, 1], f32, tag="red3")
                    nc.vector.tensor_reduce(out=red3, in_=ssq4[:, 0:3],
                                            axis=Ax.X, op=Alu.add)
                    m = sm.tile([32, 1], f32, tag="m")
                    nc.vector.tensor_scalar_mul(m, ps_sums[:, 0:1], 1.0 / _ZR)
                    m2 = sm.tile([32, 1], f32, tag="m2")
                    nc.vector.tensor_mul(m2, m, m)
                    nc.vector.tensor_scalar_add(m2, m2, -_EPS)
                    vest = sm.tile([32, 1], f32, tag="vest")
                    nc.vector.scalar_tensor_tensor(
                        out=vest, in0=red3, scalar=1.0 / 1536.0, in1=m2,
                        op0=Alu.mult, op1=Alu.subtract)
                    y, yt = newton_seed(sm, 32, vest, "zr")

            # late stats: true variance + one refining NR step
            ssq = sm.tile([32, 1], f32, tag="ssq")
            nc.vector.tensor_add(ssq, red3, ssq4[:, 3:4])
            var = sm.tile([32, 1], f32, tag="var")
            nc.vector.scalar_tensor_tensor(
                out=var, in0=ssq, scalar=1.0 / _ZR, in1=m2,
                op0=Alu.mult, op1=Alu.subtract)
            newton_refine(y, yt, var)
            rinv02 = sm.tile([32, 1], f32, tag="rinv02")
            nc.vector.tensor_scalar_mul(rinv02, y, 0.2)
            nmr02 = sm.tile([32, 1], f32, tag="nmr02")
            nc.vector.scalar_tensor_tensor(
                out=nmr02, in0=m, scalar=-0.2, in1=y,
                op0=Alu.mult, op1=Alu.mult)
            bz1m = sm.tile([32, 1], f32, tag="bz1m")
            nc.vector.tensor_scalar(bz1m, nmr02, -1.0, 1.0, Alu.mult, Alu.add)

            # r path in halves; psT tag rotates per half so transposes of the
            # second half do not wait on the first half's copy-out
            t_r = bpool.tile([32, _H], f32, tag="t_r")
            s_r = bpool.tile([32, _H], f32, tag="s_r")
            rh = bpool.tile([32, _H], bf16, tag="rh")
            rhT = [bpool.tile([128, 4, 32], bf16, tag=f"rhT{h}", name=f"rhT{h}")
                   for h in range(2)]
            for hf in range(2):
                sl = slice(hf * 512, (hf + 1) * 512)
                s1sl = slice(_H + hf * 512, _H + (hf + 1) * 512)
                if affine:
                    nc.vector.tensor_scalar(t_r[:, sl], zb[2 + hf], rinv02,
                                            nmr02, Alu.mult, Alu.add)
                    nc.vector.tensor_mul(t_r[:, sl], t_r[:, sl],
                                         g1_bc[:, s1sl])
                    nc.vector.tensor_add(t_r[:, sl], t_r[:, sl], s1t[:, s1sl])
                else:
                    nc.vector.scalar_tensor_tensor(
                        out=t_r[:, sl], in0=zb[2 + hf], scalar=rinv02,
                        in1=s1t[:, s1sl], op0=Alu.mult, op1=Alu.add)
                nc.scalar.activation(out=s_r[:, sl], in_=t_r[:, sl],
                                     func=Act.Relu,
                                     bias=(0.0 if affine else nmr02))
            for hf in range(2):
                sl = slice(hf * 512, (hf + 1) * 512)
                nc.vector.scalar_tensor_tensor(
                    out=rh[:, sl], in0=s_r[:, sl], scalar=1.0,
                    in1=h_bf[:, sl], op0=Alu.min, op1=Alu.mult)
                psT = t_ps.tile([128, 4, 32], bf16, tag="T")
                for k in range(4):
                    kk = hf * 4 + k
                    nc.tensor.transpose(psT[:, k],
                                        rh[:, kk * 128:(kk + 1) * 128],
                                        ident_b)
                nc.vector.tensor_copy(out=rhT[hf], in_=psT)

            # z path: t_z shares the s_r slot so it is ordered after rh,
            # keeping the compile-time scheduler off the critical path.
            t_z = bpool.tile([32, _H], f32, tag="s_r")
            for hf in range(2):
                sl = slice(hf * 512, (hf + 1) * 512)
                if affine:
                    nc.vector.tensor_scalar(t_z[:, sl], zb[hf], rinv02,
                                            nmr02, Alu.mult, Alu.add)
                    nc.vector.tensor_mul(t_z[:, sl], t_z[:, sl],
                                         g1_bc[:, sl])
                    nc.vector.tensor_add(t_z[:, sl], t_z[:, sl], s1t[:, sl])
                else:
                    nc.vector.scalar_tensor_tensor(
                        out=t_z[:, sl], in0=zb[hf], scalar=rinv02,
                        in1=s1t[:, sl], op0=Alu.mult, op1=Alu.add)
            omz = bpool.tile([32, _H], f32, tag="omz")
            nc.scalar.activation(out=omz, in_=t_z, func=Act.Relu,
                                 scale=-1.0, bias=(1.0 if affine else bz1m))
            s_z = bpool.tile([32, _H], f32, tag="s_z")
            nc.scalar.activation(out=s_z, in_=t_z, func=Act.Relu,
                                 bias=(0.0 if affine else nmr02))
            t1 = bpool.tile([32, _H], f32, tag="t_r")
            nc.vector.scalar_tensor_tensor(
                out=t1, in0=s_z, scalar=1.0, in1=h_sb,
                op0=Alu.min, op1=Alu.mult)

            # candidate matmuls: c0, row-sums (mean early), c1
            ssq2 = sm.tile([32, 2], f32, tag="ssq2")
            for k in range(8):
                nc.tensor.matmul(ps_c0, rhT[k // 4][:, k % 4],
                                 u_sb[:, k, _ZR:_ZR + 512],
                                 start=(k == 0), stop=(k == 7))
            for k in range(8):
                nc.tensor.matmul(ps_sums[:, 1:2], rhT[k // 4][:, k % 4],
                                 u_sb[:, k, _IN3 + 1:_IN3 + 2],
                                 start=(k == 0), stop=(k == 7))
            nc.scalar.activation(out=sq_scr, in_=ps_c0, func=Act.Square,
                                 accum_out=ssq2[:, 0:1])
            # partial c-stats under the c1 matmuls
            m_c = sm.tile([32, 1], f32, tag="mc")
            nc.vector.tensor_scalar_mul(m_c, ps_sums[:, 1:2], 1.0 / _H)
            m2_c = sm.tile([32, 1], f32, tag="m2c")
            nc.vector.tensor_mul(m2_c, m_c, m_c)
            nc.vector.tensor_scalar_add(m2_c, m2_c, -_EPS)
            vest_c = sm.tile([32, 1], f32, tag="vestc")
            nc.vector.scalar_tensor_tensor(
                out=vest_c, in0=ssq2[:, 0:1], scalar=1.0 / 512.0, in1=m2_c,
                op0=Alu.mult, op1=Alu.subtract)
            y_c, yt_c = newton_seed(sm, 32, vest_c, "c")
            for k in range(8):
                nc.tensor.matmul(ps_c1, rhT[k // 4][:, k % 4],
                                 u_sb[:, k, _ZR + 512:_ZR + 1024],
                                 start=(k == 0), stop=(k == 7))
            nc.scalar.activation(out=sq_scr, in_=ps_c1, func=Act.Square,
                                 accum_out=ssq2[:, 1:2])
            # phase-A matmuls fill the PE gap while the candidate tail runs
            a_done = a_emit_chunks_pe(_AMM_PER_STEP)

            ssq_c = sm.tile([32, 1], f32, tag="ssqc")
            nc.vector.tensor_reduce(out=ssq_c, in_=ssq2, axis=Ax.X,
                                    op=Alu.add)
            var_c = sm.tile([32, 1], f32, tag="varc")
            nc.vector.scalar_tensor_tensor(
                out=var_c, in0=ssq_c, scalar=1.0 / _H, in1=m2_c,
                op0=Alu.mult, op1=Alu.subtract)
            newton_refine(y_c, yt_c, var_c)
            nmr_c = sm.tile([32, 1], f32, tag="nmrc")
            nc.vector.scalar_tensor_tensor(
                out=nmr_c, in0=m_c, scalar=-1.0, in1=y_c,
                op0=Alu.mult, op1=Alu.mult)

            # candidate tail + h update in halves
            t_c = bpool.tile([32, _H], f32, tag="t_c")
            cand = bpool.tile([32, _H], f32, tag="cand")
            t2 = bpool.tile([32, _H], f32, tag="t2")
            for hf in range(2):
                sl = slice(hf * 512, (hf + 1) * 512)
                s1sl = slice(_ZR + hf * 512, _ZR + (hf + 1) * 512)
                ps_ch = ps_c0 if hf == 0 else ps_c1
                if affine:
                    nc.vector.tensor_scalar(t_c[:, sl], ps_ch, y_c, nmr_c,
                                            Alu.mult, Alu.add)
                    nc.vector.tensor_mul(t_c[:, sl], t_c[:, sl],
                                         g1_bc[:, s1sl])
                    nc.vector.tensor_add(t_c[:, sl], t_c[:, sl], s1t[:, s1sl])
                else:
                    nc.vector.scalar_tensor_tensor(
                        out=t_c[:, sl], in0=ps_ch, scalar=y_c,
                        in1=s1t[:, s1sl], op0=Alu.mult, op1=Alu.add)
                nc.scalar.activation(out=cand[:, sl], in_=t_c[:, sl],
                                     func=Act.Tanh,
                                     bias=(0.0 if affine else nmr_c))
            for hf in range(2):
                sl = slice(hf * 512, (hf + 1) * 512)
                nc.vector.scalar_tensor_tensor(
                    out=t2[:, sl], in0=omz[:, sl], scalar=1.0,
                    in1=cand[:, sl], op0=Alu.min, op1=Alu.mult)
                nc.vector.tensor_add(h_bf[:, sl], t1[:, sl], t2[:, sl])
                psT = t_ps.tile([128, 4, 32], bf16, tag="T")
                for k in range(4):
                    kk = hf * 4 + k
                    nc.tensor.transpose(psT[:, k],
                                        h_bf[:, kk * 128:(kk + 1) * 128],
                                        ident_b)
                nc.vector.tensor_copy(out=(hT_a if hf == 0 else hT_b),
                                      in_=psT)
            nc.gpsimd.tensor_add(h_sb, t1, t2)
            nc.sync.dma_start(out=o_d.ap()[:, t, :], in_=h_sb)

            # phase-A staging (Act) after the tanh ops; throttled applies
            a_emit_chunks_act(a_done)
            a_emit_applies(_APPLY_PER_STEP)

        # drain any leftover phase-A work (should be empty)
        a_emit_chunks_act(a_emit_chunks_pe(10 ** 9))
        a_emit_applies(10 ** 9)

    nc.compile()
    return nc


def _get_nc(affine, has_bias, n_steps):
    key = (affine, has_bias, n_steps)
    if key not in _CACHE:
        _CACHE[key] = _build(affine, has_bias, n_steps)
    return _CACHE[key]


LAST_RESULTS = None


def kernel(x, W, U, b, gammas, betas, n_steps=_T, trace=False):
    global LAST_RESULTS
    import ml_dtypes
    from concourse.bass_utils import run_bass_kernel_spmd

    bf = ml_dtypes.bfloat16
    x = np.ascontiguousarray(np.asarray(x, dtype=np.float32))[:, :n_steps]
    W = np.asarray(W, dtype=np.float32)
    U = np.asarray(U, dtype=np.float32)
    b = np.asarray(b, dtype=np.float32)
    gammas = np.asarray(gammas, dtype=np.float32)
    betas = np.asarray(betas, dtype=np.float32)

    has_bias = bool(np.any(b != 0.0))
    affine = bool(np.any(gammas != 1.0) or np.any(betas != 0.0))

    u_ext = np.concatenate(
        [U,
         U[:, :_ZR].sum(1, keepdims=True, dtype=np.float64).astype(np.float32),
         U[:, _ZR:].sum(1, keepdims=True, dtype=np.float64).astype(np.float32)],
        axis=1).astype(bf)
    x_bf = x.astype(bf)
    w_bf = W.astype(bf)

    nc = _get_nc(affine, has_bias, n_steps)

    in_maps = []
    for c in range(_NCORES):
        m = {"x": np.ascontiguousarray(x_bf[c * _BL:(c + 1) * _BL]),
             "w": w_bf, "u": u_ext}
        if has_bias:
            m["bias"] = b
            m["bsum"] = np.array([b.sum(dtype=np.float64) / _IN3],
                                 dtype=np.float32)
        if affine:
            g0, b0 = gammas[0], betas[0]
            g1, b1 = gammas[1], betas[1]
            pas = np.concatenate([0.2 * g0[:_ZR], g0[_ZR:]])
            pab = np.concatenate(
                [0.2 * b0[:_ZR] + 0.5 + 0.2 * b1[:_ZR], b0[_ZR:] + b1[_ZR:]])
            m["pas"] = pas.astype(np.float32)
            m["pab"] = pab.astype(np.float32)
            m["g1"] = g1.astype(np.float32)
        in_maps.append(m)

    res = run_bass_kernel_spmd(nc, in_maps, list(range(_NCORES)), trace=trace)
    LAST_RESULTS = res
    out = np.concatenate([res.results[c]["o"] for c in range(_NCORES)], axis=0)
    return out
